# revision 1
# baseline (speedup 1.0000x reference)
"""Enformer multi-head attention with central-mask relative position bias.

Trainium2 Bass/Tile kernel, sharded over 8 NeuronCores.

Problem (fp32): x [2, 1024, 768]; H=8 heads, dqk=dv=64, n_pos=64.
  q,k,v = x @ {Wq,Wk,Wv}.T ; basis[i,j,:] = f(j-i)  (Toeplitz!)
  qr = (q @ w_pos) . basis ; uk = u.k ; vr = (v_bias.w_pos) . basis
  scores = (q.k + qr + uk + vr)/8 ; out = softmax(scores) @ v @ Wo.T + bo

Sharding: core c owns head c for both batches (16 (b,h) units / 8 cores).
The output projection needs all heads per row, so the per-head attention
outputs avT [64, 2048] are resharded with an on-device AllToAll into
row-shards [512, 256]; each core projects its own 256 rows and returns
out_shard [256, 768]; the host concatenates.

Relative-position trick: basis[i,j,:] = B[j-i+1023, :] depends only on the
diagonal, so qr[i,j] + vr[i,j] = T'[i, j-i+1023] with
T' = (qw + vw) @ B.T  ([1024, 2047] per (b,h)).  T' is computed in
128-row strips [128, 1152] (the window of diagonals a 128-row i-tile can
touch), bounced through DRAM, and read back with a skewed access pattern
(partition stride 1151 elements) that turns diagonals into rows.  uk[j] is
folded into the scores matmul as a 65th contraction row (q row 64 = ones,
k row 64 = uk).
"""

import sys

sys.path.insert(0, "/opt/trn_rl_repo")

import numpy as np

import concourse.bass as bass
import concourse.mybir as mybir
import concourse.tile as tile
from concourse import bacc
from concourse.bass_utils import run_bass_kernel_spmd
from concourse.masks import make_identity

N_CORES = 8
B, L, DM = 2, 1024, 768
H, DQK, DV, POS = 8, 64, 64, 64
ROWS = B * L            # 2048
SHARD = ROWS // N_CORES  # 256
NT = L // 128            # 8 i-tiles per batch
STRIP_W = 1152           # 3 matmul chunks: 512 + 512 + 128 (window is 1151)
F32 = mybir.dt.float32


def _basis_bt() -> np.ndarray:
    """B.T [64, 2048]: basis value for each signed distance d = r - 1023.

    Mirrors reference._rel_basis's float32 arithmetic; col 2047 is padding.
    """
    half = POS // 2
    d = np.arange(-(L - 1), L, dtype=np.int64)  # [2047]
    log_v = np.log(np.float32((L + 1) / 2.0)).astype(np.float32)
    pow_rate = np.exp(log_v / np.float32(half)).astype(np.float32)
    widths = (pow_rate ** np.arange(1, half + 1, dtype=np.float32)).astype(np.float32)
    unsigned = np.abs(d)[:, None].astype(np.float32) <= widths[None, :]
    signed = np.sign(d)[:, None] * unsigned
    bmat = np.concatenate(
        [unsigned.astype(np.float32), signed.astype(np.float32)], axis=1
    )  # [2047, 64]
    bt = np.zeros((POS, 2 * L), np.float32)
    bt[:, : 2 * L - 1] = bmat.T
    return bt


def _build_program():
    nc = bacc.Bacc("TRN2", target_bir_lowering=False, debug=False, num_devices=N_CORES)

    xT = nc.dram_tensor("xT", [DM, ROWS], F32, kind="ExternalInput")
    wqk = nc.dram_tensor("wqk", [DM, 2 * DQK], F32, kind="ExternalInput")
    wv = nc.dram_tensor("wv", [DM, DV], F32, kind="ExternalInput")
    wpos = nc.dram_tensor("wpos", [DQK, POS], F32, kind="ExternalInput")
    uaug = nc.dram_tensor("uaug", [DQK, DQK + 1], F32, kind="ExternalInput")
    vvec = nc.dram_tensor("vvec", [DQK, 1], F32, kind="ExternalInput")
    wo = nc.dram_tensor("wo", [H * DV, DM], F32, kind="ExternalInput")
    bo = nc.dram_tensor("bo", [1, DM], F32, kind="ExternalInput")
    out = nc.dram_tensor("out_shard", [SHARD, DM], F32, kind="ExternalOutput")

    bt_const = nc.inline_tensor(_basis_bt(), name="bt_const")

    with tile.TileContext(nc) as tc:
        _emit(nc, tc, xT, wqk, wv, wpos, uaug, vvec, wo, bo, bt_const, out)
    nc.compile()
    return nc


def _emit(nc, tc, xT, wqk, wv, wpos, uaug, vvec, wo, bo, bt_const, out):
    import contextlib

    ctx = contextlib.ExitStack()
    with ctx:
        consts = ctx.enter_context(tc.tile_pool(name="consts", bufs=1))
        perb = ctx.enter_context(tc.tile_pool(name="perb", bufs=1))
        work = ctx.enter_context(tc.tile_pool(name="work", bufs=2))
        pwork = ctx.enter_context(tc.tile_pool(name="pwork", bufs=3))
        ps_b1 = ctx.enter_context(tc.tile_pool(name="ps_b1", bufs=3, space="PSUM"))
        ps_av = ctx.enter_context(tc.tile_pool(name="ps_av", bufs=1, space="PSUM"))
        ps_sc = ctx.enter_context(tc.tile_pool(name="ps_sc", bufs=2, space="PSUM"))
        dram = ctx.enter_context(tc.tile_pool(name="dram", bufs=4, space="DRAM"))

        # ---- constants ----
        ident = consts.tile([128, 128], F32)
        make_identity(nc, ident)
        bt_sb = consts.tile([POS, 2 * L], F32)
        nc.sync.dma_start(out=bt_sb, in_=bt_const[:])
        xT_sb = consts.tile([128, 6, ROWS], F32)
        nc.sync.dma_start(out=xT_sb, in_=xT[:].rearrange("(c p) i -> p c i", p=128))
        wqk_sb = consts.tile([128, 6, 2 * DQK], F32)
        nc.sync.dma_start(out=wqk_sb, in_=wqk[:].rearrange("(c p) m -> p c m", p=128))
        wv_sb = consts.tile([128, 6, DV], F32)
        nc.sync.dma_start(out=wv_sb, in_=wv[:].rearrange("(c p) m -> p c m", p=128))
        wpos_sb = consts.tile([DQK, POS], F32)
        nc.sync.dma_start(out=wpos_sb, in_=wpos[:])
        uaug_sb = consts.tile([DQK, DQK + 1], F32)
        nc.sync.dma_start(out=uaug_sb, in_=uaug[:])
        vvec_sb = consts.tile([DQK, 1], F32)
        nc.sync.dma_start(out=vvec_sb, in_=vvec[:])
        wo_sb = consts.tile([128, 4, DM], F32)
        nc.sync.dma_start(out=wo_sb, in_=wo[:].rearrange("(c p) m -> p c m", p=128))
        bo_sb = consts.tile([1, DM], F32)
        nc.sync.dma_start(out=bo_sb, in_=bo[:])
        ones_sb = consts.tile([1, 128], F32)
        nc.vector.memset(ones_sb, 1.0)
        # attention output, d-major, column r = b*1024 + i
        avT_sb = consts.tile([DV, ROWS], F32)

        scale = 1.0 / np.sqrt(DQK)

        for b in range(B):
            base = b * L
            # ---- q/k projection -> qT_aug/kT_aug [65, 1024] (row 64 = ones / uk)
            qT_sb = perb.tile([DQK + 1, L], F32)
            kT_sb = perb.tile([DQK + 1, L], F32)
            for ch in range(2):
                cols = slice(ch * 512, (ch + 1) * 512)
                ps_qk = ps_b1.tile([128, 512], F32, tag="bank")
                for ck in range(6):
                    nc.tensor.matmul(
                        ps_qk,
                        lhsT=wqk_sb[:, ck, :],
                        rhs=xT_sb[:, ck, base + ch * 512 : base + (ch + 1) * 512],
                        start=(ck == 0),
                        stop=(ck == 5),
                    )
                nc.vector.tensor_copy(qT_sb[0:DQK, cols], ps_qk[0:DQK, :])
                nc.vector.tensor_copy(kT_sb[0:DQK, cols], ps_qk[DQK:128, :])
            nc.vector.memset(qT_sb[DQK : DQK + 1, :], 1.0)
            # uk row: lhsT = uaug (cols 0..63 zero, col 64 = u) -> row 64 = u.k
            for ch in range(2):
                cols = slice(ch * 512, (ch + 1) * 512)
                ps_uk = ps_b1.tile([DQK + 1, 512], F32, tag="bank")
                nc.tensor.matmul(
                    ps_uk, lhsT=uaug_sb, rhs=kT_sb[0:DQK, cols], start=True, stop=True
                )
                nc.vector.tensor_copy(
                    kT_sb[DQK : DQK + 1, cols], ps_uk[DQK : DQK + 1, :]
                )

            # ---- v projection, natural layout v[j, d] ----
            v_sb = perb.tile([128, NT, DV], F32)
            for jt in range(NT):
                ps_v = ps_b1.tile([128, DV], F32, tag="bank")
                for ck in range(6):
                    nc.tensor.matmul(
                        ps_v,
                        lhsT=xT_sb[:, ck, base + jt * 128 : base + (jt + 1) * 128],
                        rhs=wv_sb[:, ck, :],
                        start=(ck == 0),
                        stop=(ck == 5),
                    )
                nc.vector.tensor_copy(v_sb[:, jt, :], ps_v)

            # ---- qwT' = w_pos.T @ qT + vw (vw = w_pos.T @ v_bias) ----
            ps_vw = ps_b1.tile([POS, 1], F32, tag="bank")
            nc.tensor.matmul(ps_vw, lhsT=wpos_sb, rhs=vvec_sb, start=True, stop=True)
            vw_sb = perb.tile([POS, 1], F32)
            nc.vector.tensor_copy(vw_sb, ps_vw)
            qw_sb = perb.tile([POS, L], F32)
            for ch in range(2):
                cols = slice(ch * 512, (ch + 1) * 512)
                ps_qw = ps_b1.tile([POS, 512], F32, tag="bank")
                nc.tensor.matmul(
                    ps_qw, lhsT=wpos_sb, rhs=qT_sb[0:DQK, cols], start=True, stop=True
                )
                nc.vector.tensor_scalar_add(qw_sb[:, cols], in0=ps_qw, scalar1=vw_sb)

            # ---- per i-tile: T' strip -> DRAM -> skewed read = qr+vr tile ----
            for t in range(NT):
                s_t = 896 - 128 * t  # first diagonal index this i-tile can touch
                stage = work.tile([128, STRIP_W], F32)
                for cw, c0 in ((512, 0), (512, 512), (128, 1024)):
                    ps_st = ps_b1.tile([128, cw], F32, tag="bank", name=f"ps_st{c0}")
                    nc.tensor.matmul(
                        ps_st,
                        lhsT=qw_sb[:, t * 128 : (t + 1) * 128],
                        rhs=bt_sb[:, s_t + c0 : s_t + c0 + cw],
                        start=True,
                        stop=True,
                    )
                    nc.scalar.copy(stage[:, c0 : c0 + cw], ps_st)
                strip_d = dram.tile([128, STRIP_W], F32)
                nc.sync.dma_start(out=strip_d[:], in_=stage)
                qr_sb = work.tile([128, L], F32)
                for ch in range(2):
                    src = bass.AP(
                        tensor=strip_d.tensor,
                        offset=strip_d.offset + 127 + ch * 512,
                        ap=[[STRIP_W - 1, 128], [1, 512]],
                    )
                    nc.sync.dma_start(
                        out=qr_sb[:, ch * 512 : (ch + 1) * 512], in_=src
                    )

                # ---- scores (K=65 folds uk), + (qr+vr), exp, normalize ----
                ps_s = ps_sc.tile([128, L], F32, tag="scores")
                for ch in range(2):
                    cols = slice(ch * 512, (ch + 1) * 512)
                    nc.tensor.matmul(
                        ps_s[:, cols],
                        lhsT=qT_sb[:, t * 128 : (t + 1) * 128],
                        rhs=kT_sb[:, cols],
                        start=True,
                        stop=True,
                    )
                nc.vector.tensor_add(ps_s, ps_s, qr_sb)
                p_sb = pwork.tile([128, L], F32, tag="p")
                den = pwork.tile([128, 1], F32, tag="den")
                nc.scalar.activation(
                    out=p_sb,
                    in_=ps_s,
                    func=mybir.ActivationFunctionType.Exp,
                    scale=float(scale),
                    accum_out=den,
                )
                rden = pwork.tile([128, 1], F32, tag="rden")
                nc.vector.reciprocal(rden, den)
                nc.vector.tensor_scalar_mul(p_sb, in0=p_sb, scalar1=rden)

                # ---- attn @ v : transpose p blocks, accumulate [128, 64] ----
                ps_o = ps_av.tile([128, DV], F32, tag="av")
                for jt in range(NT):
                    ps_tp = ps_b1.tile([128, 128], F32, tag="bank")
                    nc.tensor.transpose(
                        ps_tp, p_sb[:, jt * 128 : (jt + 1) * 128], ident
                    )
                    pT_sb = pwork.tile([128, 128], F32, tag="pT")
                    nc.scalar.copy(pT_sb, ps_tp)
                    nc.tensor.matmul(
                        ps_o,
                        lhsT=pT_sb,
                        rhs=v_sb[:, jt, :],
                        start=(jt == 0),
                        stop=(jt == NT - 1),
                    )
                av_sb = pwork.tile([128, DV], F32, tag="avs")
                nc.vector.tensor_copy(av_sb, ps_o)
                ps_avt = ps_b1.tile([DV, 128], F32, tag="bank")
                nc.tensor.transpose(ps_avt, av_sb, ident)
                nc.vector.tensor_copy(
                    avT_sb[:, base + t * 128 : base + (t + 1) * 128], ps_avt
                )

        # ---- AllToAll reshard: heads -> row shards ----
        a2a_in = dram.tile([N_CORES, DV, SHARD], F32, tag="a2a_in")
        a2a_out = dram.tile([N_CORES, DV, SHARD], F32, tag="a2a_out")
        nc.sync.dma_start(
            out=a2a_in[:].rearrange("s d i -> d s i"),
            in_=avT_sb[:].rearrange("d (s i) -> d s i", s=N_CORES),
        )
        nc.gpsimd.collective_compute(
            "AllToAll",
            mybir.AluOpType.bypass,
            replica_groups=[list(range(N_CORES))],
            ins=[a2a_in.opt()],
            outs=[a2a_out.opt()],
        )
        avall_sb = consts.tile([128, 4, SHARD], F32)
        nc.sync.dma_start(
            out=avall_sb,
            in_=a2a_out[:].rearrange("s d i -> (s d) i").rearrange(
                "(c p) i -> p c i", p=128
            ),
        )

        # ---- output projection on own 256 rows: [256, 512] @ [512, 768] + bo
        for it in range(SHARD // 128):
            ps_proj = ps_sc.tile([128, DM], F32, tag="scores")
            for cols in (slice(0, 512), slice(512, DM)):
                for cc in range(4):
                    nc.tensor.matmul(
                        ps_proj[:, cols],
                        lhsT=avall_sb[:, cc, it * 128 : (it + 1) * 128],
                        rhs=wo_sb[:, cc, cols],
                        start=(cc == 0),
                        stop=False,
                    )
                nc.tensor.matmul(
                    ps_proj[:, cols],
                    lhsT=ones_sb,
                    rhs=bo_sb[:, cols],
                    start=False,
                    stop=True,
                )
            o_sb = work.tile([128, DM], F32, tag="osb")
            nc.vector.tensor_copy(o_sb, ps_proj)
            nc.sync.dma_start(out=out[it * 128 : (it + 1) * 128, :], in_=o_sb)


_PROGRAM = None


def _get_program():
    global _PROGRAM
    if _PROGRAM is None:
        _PROGRAM = _build_program()
    return _PROGRAM


def _in_maps(x, Wq, Wk, Wv, Wo, bo, u_bias, v_bias, w_pos):
    xT = np.ascontiguousarray(x.reshape(ROWS, DM).T).astype(np.float32)
    woT = np.ascontiguousarray(Wo.T).astype(np.float32)
    bo_row = np.ascontiguousarray(bo[None, :]).astype(np.float32)
    maps = []
    for h in range(N_CORES):
        sl = slice(h * DQK, (h + 1) * DQK)
        wqk = np.concatenate([Wq[sl].T, Wk[sl].T], axis=1)
        uaug = np.zeros((DQK, DQK + 1), np.float32)
        uaug[:, DQK] = u_bias[h]
        maps.append(
            {
                "xT": xT,
                "wqk": np.ascontiguousarray(wqk).astype(np.float32),
                "wv": np.ascontiguousarray(Wv[sl].T).astype(np.float32),
                "wpos": np.ascontiguousarray(w_pos[h]).astype(np.float32),
                "uaug": uaug,
                "vvec": np.ascontiguousarray(v_bias[h][:, None]).astype(np.float32),
                "wo": woT,
                "bo": bo_row,
            }
        )
    return maps


def kernel(x, Wq, Wk, Wv, Wo, bo, u_bias, v_bias, w_pos, _trace=False):
    nc = _get_program()
    maps = _in_maps(
        np.asarray(x), np.asarray(Wq), np.asarray(Wk), np.asarray(Wv),
        np.asarray(Wo), np.asarray(bo), np.asarray(u_bias), np.asarray(v_bias),
        np.asarray(w_pos),
    )
    res = run_bass_kernel_spmd(
        nc, maps, core_ids=list(range(N_CORES)), trace=_trace
    )
    full = np.concatenate(
        [res.results[c]["out_shard"] for c in range(N_CORES)], axis=0
    )
    if _trace:
        kernel.last_exec_time_ns = res.exec_time_ns
        kernel.last_results = res
    return full.reshape(B, L, DM)
